# revision 20
# baseline (speedup 1.0000x reference)
"""GAT message-passing layer on 8 Trainium2 NeuronCores (Bass/Tile).

Sharding: data-parallel over batch (4 graphs) x 2-way edge partition by
target node within each graph -> 8 cores, fully independent (no collectives).

Per core:
  phase 1: proj = x @ W^T (bf16) for all nodes of its graph -> HBM scratch
           (PE transpose of x tiles + matmul against W^T).
  phase 2: edges sorted by target node, grouped into 40 node-tiles
           (128 targets each) x P_G groups of 128 edges (host-padded).
           Per tile: dma_gather proj[src] rows (bf16), one batched one-hot
           scatter matrix S[p,g,j] = (trg_local[p,g]==j) via is_equal,
           PSUM-accumulated matmuls S^T @ [proj_src*exp | exp] and S^T @ rel.
           Finalize: att = acc/(denom+1e-16), rel_sum @ W^T (PE transpose
           + matmul), + skip + bias, ELU (fp32), write out.

The reference's global-max subtraction inside softmax is dropped: softmax
is invariant to it except through the +1e-16 epsilon, a ~1e-12 relative
perturbation here (scores are O(5), so exp() cannot overflow).

Host-side prep: edge sort/padding/index layouts, rel row permutation (cast
bf16), and the per-edge pre-activation scores u = s_src[src]+s_trg[trg]
where s_* = x @ (W^T a_*) in fp32 (a tiny folded-weight matmul, 0.6% of
total FLOPs).
"""

import numpy as np
import ml_dtypes

import concourse.bass as bass
import concourse.tile as tile
from concourse import mybir, bacc
from concourse.bass_utils import run_bass_kernel_spmd
from concourse.masks import make_identity

P = 128
B, N, E, H, F = 4, 10000, 100000, 8, 32
FIN = H * F  # 256
SPLIT = 4992  # 39 * 128; even cores own nodes [0,4992), odd [4992,10000)
NT = 40  # node tiles per core (capacity NT*128 = 5120 rows)
NPAD = 10240  # padded node count for proj scratch (80 tiles)
F32 = mybir.dt.float32
BF16 = mybir.dt.bfloat16
NPBF = ml_dtypes.bfloat16

TRACE = False  # test.py can flip this for profiling


def build_program(P_G: int):
    """Build the SPMD program (identical on all 8 cores)."""
    G = NT * P_G  # total 128-edge groups per core
    EC = G * P  # padded edge capacity per core
    nc = bacc.Bacc(
        "TRN2", target_bir_lowering=False, num_devices=8, num_swdge_queues=4
    )

    # ---- external inputs (bound per core) ----
    x_full = nc.declare_dram_parameter("x_full", [NPAD, FIN], BF16, isOutput=False)
    w_t = nc.declare_dram_parameter("w_t", [FIN, FIN], BF16, isOutput=False)  # W^T [k,o]
    x_skip = nc.declare_dram_parameter("x_skip", [NT * P, FIN], F32, isOutput=False)
    u_e = nc.declare_dram_parameter("u_e", [P, G * H], F32, isOutput=False)
    trg_l = nc.declare_dram_parameter("trg_l", [P, G], BF16, isOutput=False)
    sidx = nc.declare_dram_parameter("sidx", [P, EC // 16], mybir.dt.int16, isOutput=False)
    iota_in = nc.declare_dram_parameter("iota_in", [P, P], BF16, isOutput=False)
    rel_s = nc.declare_dram_parameter("rel_s", [P, G * FIN], BF16, isOutput=False)
    out_c = nc.declare_dram_parameter("out_c", [NT * P, FIN], F32, isOutput=True)


    with tile.TileContext(nc) as tc:
        with (
            tc.tile_pool(name="dram", bufs=1, space="DRAM") as dpool,
            tc.tile_pool(name="const", bufs=1) as cpool,
            tc.tile_pool(name="resi", bufs=1) as rpool,
            tc.tile_pool(name="p1", bufs=3) as p1,
            tc.tile_pool(name="p2", bufs=3) as p2,
            tc.tile_pool(name="fin", bufs=2) as fin,
            tc.tile_pool(name="ps", bufs=2, space="PSUM") as ps,
            tc.tile_pool(name="ps1", bufs=2, space="PSUM") as ps1,
        ):
            proj_d = dpool.tile([NPAD, FIN], BF16)
            XB = 4
            x4 = x_full[:].rearrange("(n a p) k -> n p a k", a=XB, p=P)
            pj4 = proj_d[:].rearrange("(n a p) k -> n p a k", a=XB, p=P)

            identb = cpool.tile([P, P], BF16)
            make_identity(nc, identb[:])
            iota_s = cpool.tile([P, P], BF16)
            nc.sync.dma_start(iota_s[:], iota_in[:])
            wt_s = cpool.tile([P, 2, FIN], BF16)  # [k%128, k//128, o]
            nc.sync.dma_start(wt_s[:], w_t[:].rearrange("(a p) o -> p a o", p=P))

            # resident per-core edge metadata
            u_s = rpool.tile([P, G * H], F32)
            nc.scalar.dma_start(u_s[:], u_e[:])
            trg_s = rpool.tile([P, G], BF16)
            nc.sync.dma_start(trg_s[:], trg_l[:])
            sidx_s = rpool.tile([P, EC // 16], mybir.dt.int16)
            nc.scalar.dma_start(sidx_s[:], sidx[:])


            # ---- phase 1: proj = x @ W^T (bf16) ----
            # 8 transposes into ONE psum bank, one DVE copy, 8 dense matmuls
            for n4 in range(NPAD // (XB * P)):
                xt = p1.tile([P, XB, FIN], BF16, tag="xt")
                nc.sync.dma_start(xt[:], x4[n4])
                tp = ps1.tile([P, XB, FIN], BF16, space="PSUM", tag="tp")
                for a in range(XB):
                    nc.tensor.transpose(tp[:, a, 0:P], xt[:, a, 0:P], identb[:])
                    nc.tensor.transpose(tp[:, a, P:FIN], xt[:, a, P:FIN], identb[:])
                xT = p1.tile([P, XB, FIN], BF16, tag="xT")
                nc.vector.tensor_copy(xT[:], tp[:])
                pout = p1.tile([P, XB, FIN], BF16, tag="pout")
                for a in range(XB):
                    pp = ps1.tile([P, FIN], F32, space="PSUM", tag="mm")
                    nc.tensor.matmul(
                        pp[:], lhsT=xT[:, a, 0:P], rhs=wt_s[:, 0, :],
                        start=True, stop=False,
                    )
                    nc.tensor.matmul(
                        pp[:], lhsT=xT[:, a, P:FIN], rhs=wt_s[:, 1, :],
                        start=False, stop=True,
                    )
                    nc.scalar.copy(pout[:, a, :], pp[:])
                nc.sync.dma_start(pj4[n4], pout[:])

            # ---- phase 2: edge aggregation, two node tiles per step ----
            # pairs batch the gathers (3x1024 packed calls), the DVE/ACT ops
            # (over 2*P_G groups), and keep PE bursts long (HAM stays warm).
            # finalize(prev pair) is emitted after front(pair) so the DVE
            # never head-of-line blocks on the accumulation matmuls.
            IW = P_G * P // 16  # sidx columns per tile
            G2 = 2 * P_G
            pend = {}  # t -> (ad, rp)

            def front(pr):
                tA = 2 * pr
                pg = p2.tile([P, G2, FIN], BF16, tag="pg")
                base_col = tA * IW
                gchunks = list(range(0, G2, 8))
                for j, ga in enumerate(gchunks):
                    gb = min(ga + 8, G2)
                    nh = (gb - ga) * P
                    nc.gpsimd.dma_gather(
                        pg[:, ga:gb, :],
                        proj_d[:],
                        sidx_s[:, base_col + ga * 8:base_col + gb * 8],
                        num_idxs=nh,
                        num_idxs_reg=nh,
                        elem_size=FIN,
                        single_packet=(nh <= 1024),
                        queue_num=(len(gchunks) * pr + j) % 4,
                    )
                rl = p2.tile([P, G2 * FIN], BF16, tag="rl")
                nc.sync.dma_start(
                    rl[:], rel_s[:, tA * P_G * FIN:(tA + 2) * P_G * FIN]
                )

                # scores -> exp (fp32 in, bf16 out)
                lr = p2.tile([P, G2 * H], F32, tag="lr")
                ut = u_s[:, tA * P_G * H:(tA + 2) * P_G * H]
                nc.vector.tensor_scalar_mul(lr[:], ut, 0.2)
                nc.vector.tensor_tensor(lr[:], lr[:], ut, op=mybir.AluOpType.max)
                ex = p2.tile([P, G2 * H], BF16, tag="ex")
                nc.scalar.activation(ex[:], lr[:], mybir.ActivationFunctionType.Exp)
                # expanded exp (each head value repeated F times), written by ACT
                exr = p2.tile([P, G2 * H, F], BF16, tag="exr")
                nc.scalar.activation(
                    exr[:],
                    lr[:].unsqueeze(2).broadcast_to([P, G2 * H, F]),
                    mybir.ActivationFunctionType.Exp,
                )

                # batched one-hot S for all groups of both tiles
                S_all = p2.tile([P, G2, P], BF16, tag="S")
                nc.vector.tensor_tensor(
                    S_all[:],
                    iota_s[:].unsqueeze(1).broadcast_to([P, G2, P]),
                    trg_s[:, tA * P_G:(tA + 2) * P_G].unsqueeze(2)
                    .broadcast_to([P, G2, P]),
                    op=mybir.AluOpType.is_equal,
                )

                # msg_all[:, g, 0:256] = proj_src * exp ; [:, g, 256:264] = exp
                ma = p2.tile([P, G2, FIN + H], BF16, tag="ma")
                nc.vector.tensor_tensor(
                    ma[:, :, 0:FIN],
                    pg[:],
                    exr[:].rearrange("p (g h) f -> p g (h f)", h=H),
                    op=mybir.AluOpType.mult,
                )
                nc.scalar.copy(
                    ma[:, :, FIN:FIN + H],
                    ex[:].rearrange("p (g h) -> p g h", h=H),
                )

                for tt in range(2):
                    ad = ps.tile([P, FIN + H], F32, space="PSUM", tag="ad")
                    rp = ps.tile([P, FIN], F32, space="PSUM", tag="rp")
                    for g in range(P_G):
                        gg = tt * P_G + g
                        nc.tensor.matmul(
                            ad[:], lhsT=S_all[:, gg, :], rhs=ma[:, gg, :],
                            start=(g == 0), stop=(g == P_G - 1),
                        )
                        nc.tensor.matmul(
                            rp[:], lhsT=S_all[:, gg, :],
                            rhs=rl[:, gg * FIN:(gg + 1) * FIN],
                            start=(g == 0), stop=(g == P_G - 1),
                        )
                    pend[tA + tt] = (ad, rp)

            def finalize(t):
                ad, rp = pend.pop(t)
                rcp = fin.tile([P, H], F32, tag="rcp")
                nc.vector.tensor_scalar_add(rcp[:], ad[:, FIN:FIN + H], 1e-16)
                nc.vector.reciprocal(rcp[:], rcp[:])
                att = fin.tile([P, FIN], F32, tag="att")
                nc.vector.tensor_tensor(
                    att[:].rearrange("p (h f) -> p h f", h=H),
                    ad[:, 0:FIN].rearrange("p (h f) -> p h f", h=H),
                    rcp[:].unsqueeze(2).broadcast_to([P, H, F]),
                    op=mybir.AluOpType.mult,
                )
                # rel_sum @ W^T: transpose rel_sum (bf16), then matmul with W^T
                rsb = fin.tile([P, FIN], BF16, tag="rsb")
                nc.scalar.copy(rsb[:], rp[:])
                rt = ps1.tile([P, FIN], BF16, space="PSUM", tag="tp")
                nc.tensor.transpose(rt[:, 0:P], rsb[:, 0:P], identb[:])
                nc.tensor.transpose(rt[:, P:FIN], rsb[:, P:FIN], identb[:])
                rts = fin.tile([P, FIN], BF16, tag="rts")
                nc.scalar.copy(rts[:], rt[:])
                po = ps1.tile([P, FIN], F32, space="PSUM", tag="mm")
                nc.tensor.matmul(
                    po[:], lhsT=rts[:, 0:P], rhs=wt_s[:, 0, :],
                    start=True, stop=False,
                )
                nc.tensor.matmul(
                    po[:], lhsT=rts[:, P:FIN], rhs=wt_s[:, 1, :],
                    start=False, stop=True,
                )
                # combine + ELU (fp32): elu(x) = max(x, exp(-relu(-x)) - 1)
                xsk_t = fin.tile([P, FIN], F32, tag="xsk")
                nc.scalar.dma_start(xsk_t[:], x_skip[t * P:(t + 1) * P, :])
                comb = fin.tile([P, FIN], F32, tag="comb")
                nc.vector.tensor_tensor(comb[:], att[:], po[:], op=mybir.AluOpType.add)
                nc.vector.tensor_tensor(
                    comb[:], comb[:], xsk_t[:], op=mybir.AluOpType.add
                )
                en = fin.tile([P, FIN], F32, tag="en")
                nc.scalar.activation(
                    en[:], comb[:], mybir.ActivationFunctionType.Relu, scale=-1.0
                )
                nc.scalar.activation(
                    en[:], en[:], mybir.ActivationFunctionType.Exp, scale=-1.0
                )
                nc.vector.tensor_scalar_add(en[:], en[:], -1.0)
                nc.vector.tensor_tensor(en[:], en[:], comb[:], op=mybir.AluOpType.max)
                nc.sync.dma_start(out_c[t * P:(t + 1) * P, :], en[:])

            NPAIR = NT // 2
            for pr in range(NPAIR):
                front(pr)
                if pr >= 1:
                    finalize(2 * pr - 2)
                    finalize(2 * pr - 1)
            finalize(NT - 2)
            finalize(NT - 1)

    nc.compile()
    return nc


def _prep_core(x_b, src, trg, rel_b, s_src, s_trg, bias, half, P_G):
    """Build one core's input arrays."""
    G = NT * P_G
    EC = G * P
    base = 0 if half == 0 else SPLIT
    m = (trg < SPLIT) if half == 0 else (trg >= SPLIT)
    src_h, trg_h = src[m], trg[m]
    order = np.argsort(trg_h, kind="stable")
    src_h, trg_h = src_h[order], trg_h[order]
    eid = np.nonzero(m)[0][order]  # original edge ids, sorted by trg

    tile_of = (trg_h - base) // P
    counts = np.bincount(tile_of, minlength=NT)
    assert counts.max() <= P_G * P, (counts.max(), P_G * P)

    # slot j in [0, EC): tile t = j // (P_G*P); within tile, edge i sits at
    # partition i%128, group i//128 (matches dma_gather output layout)
    src_pad = np.zeros(EC, dtype=np.int64)
    trg_pad = np.full(EC, 200.0, dtype=np.float32)  # sentinel > 127
    u_pad = np.zeros((EC, H), dtype=np.float32)
    eid_pad = np.zeros(EC, dtype=np.int64)  # pads gather rel row 0 (killed by S)

    starts = np.concatenate([[0], np.cumsum(counts)])[:-1]
    pos_in_tile = np.arange(len(trg_h)) - starts[tile_of]
    slot = tile_of * (P_G * P) + pos_in_tile
    src_pad[slot] = src_h
    trg_pad[slot] = (trg_h - base) % P
    u_pad[slot] = s_src[src_h] + s_trg[trg_h]
    eid_pad[slot] = eid

    def to_pg(a):  # [EC, ...] -> [P, G, ...]; [p, t*P_G+g] = slot t*P_G*P+g*P+p
        a = a.reshape(NT, P_G, P, *a.shape[1:])
        a = np.moveaxis(a, 2, 0)  # [P, NT, P_G, ...]
        return np.ascontiguousarray(a.reshape(P, G, *a.shape[3:]))

    u_arr = to_pg(u_pad).reshape(P, G * H).astype(np.float32)
    trg_arr = to_pg(trg_pad).reshape(P, G).astype(NPBF)
    rel_arr = to_pg(rel_b[eid_pad].astype(NPBF)).reshape(P, G * FIN)

    # gather indices: per-tile call, idx i_loc -> [i_loc%16, i_loc//16], x8 replicas
    si = src_pad.reshape(NT, P_G * P)
    cols = P_G * P // 16
    si16 = np.zeros((16, NT, cols), dtype=np.int16)
    i_loc = np.arange(P_G * P)
    for t in range(NT):
        si16[i_loc % 16, t, i_loc // 16] = si[t]
    sidx_arr = np.tile(si16.reshape(16, NT * cols), (8, 1))

    xsk = np.zeros((NT * P, FIN), dtype=np.float32)
    nvalid = min(NT * P, N - base)
    xsk[:nvalid] = x_b[base:base + nvalid]
    xsk += bias[None, :]

    return dict(x_skip=xsk, u_e=u_arr, trg_l=trg_arr, sidx=sidx_arr,
                rel_s=rel_arr)


_CACHE = {}


def kernel(x, edge_index, rel, W_proj, a_src, a_trg, bias, **_ignored):
    x = np.asarray(x, dtype=np.float32)
    edge_index = np.asarray(edge_index)
    rel = np.asarray(rel, dtype=np.float32)
    W_proj = np.asarray(W_proj, dtype=np.float32)
    a_src = np.asarray(a_src, dtype=np.float32)
    a_trg = np.asarray(a_trg, dtype=np.float32)
    bias = np.asarray(bias, dtype=np.float32)

    # folded score weights: s_* = x @ A_*, A_*[k,h] = sum_f W[h*F+f,k] a_*[h,f]
    Wr = W_proj.reshape(H, F, FIN)
    A_src = np.einsum("hfk,hf->kh", Wr, a_src[0]).astype(np.float32)
    A_trg = np.einsum("hfk,hf->kh", Wr, a_trg[0]).astype(np.float32)

    # global P_G (max groups per 128-node tile over all cores)
    maxc = 0
    for b in range(B):
        trg = np.asarray(edge_index[b, 1], dtype=np.int64)
        for half in range(2):
            base = 0 if half == 0 else SPLIT
            m = (trg < SPLIT) if half == 0 else (trg >= SPLIT)
            if m.any():
                t_of = (trg[m] - base) // P
                maxc = max(maxc, int(np.bincount(t_of, minlength=NT).max()))
    P_G = max(1, -(-maxc // P))

    if P_G not in _CACHE:
        _CACHE[P_G] = build_program(P_G)
    nc = _CACHE[P_G]

    w_t = np.ascontiguousarray(W_proj.T).astype(NPBF)
    iota_np = np.tile(np.arange(P, dtype=np.float32), (P, 1)).astype(NPBF)

    in_maps = []
    for c in range(8):
        b, half = c // 2, c % 2
        src = np.asarray(edge_index[b, 0], dtype=np.int64)
        trg = np.asarray(edge_index[b, 1], dtype=np.int64)
        s_src = (x[b] @ A_src).astype(np.float32)
        s_trg = (x[b] @ A_trg).astype(np.float32)
        d = _prep_core(x[b], src, trg, rel[b], s_src, s_trg, bias, half, P_G)
        xf = np.zeros((NPAD, FIN), dtype=NPBF)
        xf[:N] = x[b].astype(NPBF)
        d.update(x_full=xf, w_t=w_t, iota_in=iota_np)
        in_maps.append(d)

    res = run_bass_kernel_spmd(nc, in_maps, core_ids=list(range(8)), trace=TRACE)
    kernel.last_result = res

    out = np.empty((B, N, FIN), dtype=np.float32)
    for c in range(8):
        b, half = c // 2, c % 2
        oc = res.results[c]["out_c"]
        if half == 0:
            out[b, :SPLIT] = oc[:SPLIT]
        else:
            out[b, SPLIT:] = oc[:N - SPLIT]
    return out
